# revision 53
# baseline (speedup 1.0000x reference)
"""Trainium2 Bass kernel for binarized 3x3 conv + batch-norm (BinConv2d).

Reference computation:
    xb = sign(x); wb = sign(weight)
    y  = conv2d(xb, wb, stride 1, pad 1)        # NCHW / OIHW
    out = batchnorm(y, batch stats over (N,H,W), affine gamma/beta)

Strategy: data-parallel over batch (64 images -> 8 images per NeuronCore).
The conv runs as shifted matmuls with Cin=128 on the SBUF partition dim,
accumulating in PSUM. Weights arrive host-signed as fp8 (+/-1 exact);
activations are signed on ACT into 3 rotating zero-padded 58x58 fp8
images. The 3x3 taps are 4 DoubleRow pairs + 1 single matmul per output
tile (~1.8x TensorE throughput vs bf16), with exact 8-row x 56-col moving
APs so PSUM tiles are contiguous 448-col runs. Image 0 loads in small
chunks (first one 10 rows) and runs tile-major so matmuls start ~11us in;
later images flood-prefetch on a 10-buf staging pool. Conv outputs are
integers |y| <= 1152: exact in fp32 PSUM and the fp16 SBUF copy. Channel
stats come from DVE bn_stats/bn_aggr (split 49/7 so the bulk aggregates
during image 7, whose stats read PSUM directly); [mean/8, E[y^2]/8] is
AllGathered (single-step mesh, cheaper than an AllReduce ring) and summed
with two DVE reduces; the affine is applied on-device before the f32
output DMA.
"""
import numpy as np

import concourse.bacc as bacc
import concourse.tile as tile
import concourse.mybir as mybir
import concourse.bass_utils as bass_utils
from concourse.bass_types import AP

F32 = mybir.dt.float32
F16 = mybir.dt.float16
F8 = mybir.dt.float8e4
AF = mybir.ActivationFunctionType
ALU = mybir.AluOpType
DR = mybir.MatmulPerfMode.DoubleRow

N_CORES = 8
N_FULL = 64            # total batch
NIMG = N_FULL // N_CORES   # images per core
C = 128                # channels (in == out)
H = W = 56
WP = W + 2             # padded width (58)
HP = H + 2             # padded height (58)
PSTRIDE = HP * WP      # per-partition elements of one image tile (3364)
NT = 7                 # row tiles per image
RT = H // NT           # rows per tile (8)
TW = RT * W            # psum tile free size (448)
IMG = H * W            # 3136
EPS = 1e-5
HH = H // 2            # DMA chunk rows (28)

TRACE = False          # test.py may flip this to get an NTFF profile

_CACHE = {}


def _build(use_collective=True, nimg=NIMG):
    nc = bacc.Bacc("TRN2", target_bir_lowering=False, debug=False,
                   num_devices=N_CORES)
    x = nc.dram_tensor("x", [NIMG, C, H, W], F32, kind="ExternalInput").ap()
    # host passes sign(weight) pre-packed as fp8 (+/-1 exact): 147KB DMA,
    # no on-device weight sign on the startup critical path
    wt = nc.dram_tensor("wt", [C, 9, C], F8, kind="ExternalInput").ap()
    gb = nc.dram_tensor("gb", [C, 2], F32, kind="ExternalInput").ap()
    out = nc.dram_tensor("out", [NIMG, C, H, W], F32, kind="ExternalOutput").ap()

    with tile.TileContext(nc) as tc:
        with tc.tile_pool(name="const", bufs=1) as pc, \
             tc.tile_pool(name="fstage", bufs=10) as pfs, \
             tc.tile_pool(name="ostage", bufs=8) as pos, \
             tc.tile_pool(name="psum", bufs=8, space="PSUM") as pp, \
             tc.tile_pool(name="dram", bufs=1, space="DRAM") as pd:

            # ---- persistent buffers ----
            y16 = pc.tile([C, NIMG, H, W], F16)       # conv ints (exact)
            bnbuf = pc.tile([C, nimg * NT, 6], F32)
            epst = pc.tile([C, 1], F32)
            wb = pc.tile([C, 9, C], F8)
            gbt = pc.tile([C, 2], F32)
            # 3 rotating padded fp8 images; pads zeroed once below
            xps = [pc.tile([C, HP, WP], F8, name=f"xp{i}")
                   for i in range(3)]

            # ---- prologue ----
            # DMA order sets HBM arrival order: image-0 chunk A first (it
            # gates the first matmul), then the tiny weight/param tensors
            fs0 = pfs.tile([C, 10, W], F32, name="fs0")
            # small first chunk: lands fast even while prefetch transfers
            # share DMA bandwidth, so the first matmuls start early
            nc.sync.dma_start(out=fs0[:], in_=x[0, :, 0:10, :])
            nc.sync.dma_start(out=wb[:], in_=wt[:])
            nc.vector.memset(epst[:], EPS)
            for xp in xps:
                nc.gpsimd.memset(xp[:, 0, :], 0.0)
                nc.gpsimd.memset(xp[:, HP - 1, :], 0.0)
                nc.gpsimd.memset(xp[:, 1:HP - 1, 0], 0.0)
                nc.gpsimd.memset(xp[:, 1:HP - 1, WP - 1], 0.0)

            def tap_off(r0, it):
                dh, dw = it // 3 - 1, it % 3 - 1
                return (r0 + 1 + dh) * WP + (1 + dw)

            def mm(xp, psums, t, p):
                r0 = t * RT
                if p < 4:
                    o0 = tap_off(r0, 2 * p)
                    o1 = tap_off(r0, 2 * p + 1)
                    rhs = AP(xp.tensor, xp.offset + o0,
                             [[PSTRIDE, C], [o1 - o0, 2], [WP, RT], [1, W]])
                    nc.tensor.matmul(out=psums[t][:],
                                     lhsT=wb[:, 2 * p:2 * p + 2, :],
                                     rhs=rhs, start=(p == 0),
                                     stop=False, perf_mode=DR)
                else:
                    o8 = tap_off(r0, 8)
                    rhs8 = AP(xp.tensor, xp.offset + o8,
                              [[PSTRIDE, C], [WP, RT], [1, W]])
                    nc.tensor.matmul(out=psums[t][:], lhsT=wb[:, 8, :],
                                     rhs=rhs8, start=False, stop=True)

            # ---- phase 1: conv + local stats, per image ----
            mvp = pc.tile([C, 2], F32)
            e1 = pc.tile([C, 1], F32)
            pbag = pc.tile([C, 2], F32)
            for n in range(nimg):
                xp = xps[n % 3]
                chunks = ((0, 10), (10, 16), (26, 15), (41, 15)) if n == 0 \
                    else ((0, HH), (HH, HH))
                for ci, (h, nr) in enumerate(chunks):
                    if n == 0 and ci == 0:
                        fs = fs0
                    else:
                        fs = pfs.tile([C, nr, W], F32, tag="fs", name="fs")
                        nc.sync.dma_start(out=fs[:], in_=x[n, :, h:h + nr, :])
                    dst = xp[:, 1 + h:1 + h + nr, 1:WP - 1]
                    if n == 0 and ci == 0:
                        # DVE sign ((x>=0)*2-1, interior-only) for the very
                        # first chunk: no ACT-table dependency, so the first
                        # matmul isn't gated on the ~1.3us ACT_TABLE_LOAD
                        nc.vector.tensor_scalar(dst, fs[:], 0.0, 2.0,
                                                ALU.is_ge, ALU.mult)
                        nc.vector.tensor_scalar_add(dst, dst, -1.0)
                    else:
                        nc.scalar.activation(out=dst, in_=fs[:],
                                             func=AF.Sign)
                    if n == 0 and ci == 0:
                        # gb is tiny and not needed until the affine;
                        # keep it behind the startup-critical transfers
                        nc.sync.dma_start(out=gbt[:], in_=gb[:])

                psums = [pp.tile([C, TW], F32, tag="ps", name="ps")
                         for _ in range(NT)]

                if n == 0 or n == nimg - 1:
                    # tile-major: image 0's tiles 0-2 only read rows of the
                    # first DMA chunk (earlier first matmul); image 7's
                    # tiles complete one by one so the stats tail is short
                    order = [(t, p) for t in range(NT) for p in range(5)]
                else:
                    # tap-major: consecutive matmuls share the stationary
                    order = [(t, p) for p in range(5) for t in range(NT)]
                for t, p in order:
                    mm(xp, psums, t, p)
                    if p == 4:
                        idx = n * NT + t
                        ydst = y16[:, n, t * RT:(t + 1) * RT, :]
                        if n == nimg - 1:
                            # last image: copies all on ACT, stats straight
                            # from PSUM on DVE, so the post-conv stats tail
                            # is just the final tile's bn_stats
                            nc.scalar.copy(out=ydst, in_=psums[t][:])
                            nc.vector.bn_stats(out=bnbuf[:, idx, :],
                                               in_=psums[t][:])
                        else:
                            # PSUM -> fp16 copy, alternating engines
                            if t % 2 == 0:
                                nc.scalar.copy(out=ydst, in_=psums[t][:])
                            else:
                                nc.vector.tensor_copy(out=ydst,
                                                      in_=psums[t][:])
                            nc.vector.bn_stats(
                                out=bnbuf[:, idx, :],
                                in_=ydst.rearrange("p r c -> p (r c)"))

                if n == 1 and use_collective:
                    # warm up the collectives firmware mid-conv (off the
                    # startup critical path) so the real AllGather's
                    # trigger latency is short
                    wbin = pd.tile([C, 1], F32)
                    wbout = pd.tile([N_CORES * C, 1], F32,
                                    addr_space="Shared")
                    nc.sync.dma_start(out=wbin[:], in_=epst[:])
                    nc.gpsimd.collective_compute(
                        "AllGather", ALU.bypass,
                        replica_groups=[list(range(N_CORES))],
                        ins=[wbin.opt()], outs=[wbout.opt()])
                    # touch Sqrt now so its ACT table loads during conv,
                    # not on the post-stats critical path
                    dum = pc.tile([C, 1], F32)
                    nc.scalar.activation(out=dum[:], in_=epst[:],
                                         func=AF.Sqrt)

                if n == nimg - 2:
                    # aggregate images 0..6 while image 7 is convolving;
                    # pre-scale its contribution to the AllReduce payload
                    nc.vector.bn_aggr(
                        out=mvp[:],
                        in_=bnbuf[:, 0:49, :].rearrange("p a s -> p (a s)"))
                    nc.vector.tensor_mul(e1[:], mvp[:, 0:1], mvp[:, 0:1])
                    nc.vector.tensor_add(e1[:], e1[:], mvp[:, 1:2])
                    nc.vector.tensor_scalar_mul(pbag[:, 0:1], mvp[:, 0:1],
                                                49.0 / (NT * N_FULL))
                    nc.vector.tensor_scalar_mul(pbag[:, 1:2], e1[:],
                                                49.0 / (NT * N_FULL))

            # ---- phase 2: finish stats, AllReduce [mean/8, E[y^2]/8] ----
            mvq = pc.tile([C, 2], F32)
            e2 = pc.tile([C, 1], F32)
            bag = pc.tile([C, 2], F32)
            nc.vector.bn_aggr(
                out=mvq[:],
                in_=bnbuf[:, 49:56, :].rearrange("p a s -> p (a s)"))
            nc.vector.tensor_mul(e2[:], mvq[:, 0:1], mvq[:, 0:1])
            nc.vector.tensor_add(e2[:], e2[:], mvq[:, 1:2])
            nc.vector.tensor_scalar_mul(bag[:, 0:1], mvq[:, 0:1],
                                        7.0 / (NT * N_FULL))
            nc.vector.tensor_scalar_mul(bag[:, 1:2], e2[:],
                                        7.0 / (NT * N_FULL))
            nc.vector.tensor_add(bag[:], bag[:], pbag[:])

            gmv = pc.tile([C, 2], F32)
            if use_collective:
                # AllGather (not AllReduce): its post-peer mesh is a single
                # broadcast step (~2us) vs the reduce ring's ~8us. Contiguous
                # [C,2] payloads both ways; the 8-way sum is two DVE reduces
                # over the strided ship-back view.
                bag_in = pd.tile([C, 2], F32)
                bag_out = pd.tile([N_CORES * C, 2], F32, addr_space="Shared")
                nc.sync.dma_start(out=bag_in[:], in_=bag[:])
                nc.gpsimd.collective_compute(
                    "AllGather", ALU.bypass,
                    replica_groups=[list(range(N_CORES))],
                    ins=[bag_in.opt()], outs=[bag_out.opt()])
                gmv8 = pc.tile([C, N_CORES, 2], F32)
                src = AP(bag_out.tensor, bag_out.offset,
                         [[2, C], [C * 2, N_CORES], [1, 2]])
                nc.sync.dma_start(out=gmv8[:], in_=src)
                nc.vector.tensor_reduce(out=gmv[:, 0:1], in_=gmv8[:, :, 0],
                                        axis=mybir.AxisListType.X, op=ALU.add)
                nc.vector.tensor_reduce(out=gmv[:, 1:2], in_=gmv8[:, :, 1],
                                        axis=mybir.AxisListType.X, op=ALU.add)
            else:
                nc.vector.tensor_scalar_mul(gmv[:], bag[:], float(N_CORES))

            # scale = gamma * rsqrt(var + eps); bias = beta - mean * scale
            # where mean = gmv[:,0], var = gmv[:,1] - mean^2
            var_t = pc.tile([C, 1], F32)
            std_t = pc.tile([C, 1], F32)
            inv_t = pc.tile([C, 1], F32)
            scale_t = pc.tile([C, 1], F32)
            bias_t = pc.tile([C, 1], F32)
            tmp_t = pc.tile([C, 1], F32)
            nc.vector.tensor_mul(var_t[:], gmv[:, 0:1], gmv[:, 0:1])
            nc.vector.tensor_sub(var_t[:], gmv[:, 1:2], var_t[:])
            nc.scalar.activation(out=std_t[:], in_=var_t[:], func=AF.Sqrt,
                                 bias=epst[:])
            nc.vector.reciprocal(inv_t[:], std_t[:])
            nc.vector.tensor_mul(scale_t[:], gbt[:, 0:1], inv_t[:])
            nc.vector.tensor_mul(tmp_t[:], gmv[:, 0:1], scale_t[:])
            nc.vector.tensor_sub(bias_t[:], gbt[:, 1:2], tmp_t[:])

            # ---- phase 3: affine + store, half-image chunks on ACT+DVE.
            # The first chunk is a 7-row sliver so the first output DMA
            # (which starts the bytes-limited drain window) issues ~0.8us
            # after scale/bias instead of waiting a full half-image affine.
            for n in range(nimg):
                ochunks = ((0, 7), (7, HH - 7), (HH, HH)) if n == 0 \
                    else ((0, HH), (HH, HH))
                for ci, (h, nr) in enumerate(ochunks):
                    ot = pos.tile([C, nr, W], F32, tag="ot", name="ot")
                    ysrc = y16[:, n, h:h + nr, :]
                    if (2 * n + ci) % 2 == 0:
                        nc.vector.tensor_scalar(
                            ot[:], ysrc, scale_t[:, 0:1], bias_t[:, 0:1],
                            ALU.mult, ALU.add)
                        # alternate trigger queues (sync / idle PE queue):
                        # two independent in-order DMA queues keep the DMA
                        # engines fed across chunk boundaries -- with one
                        # queue they sat ~17% idle between chunks
                        nc.sync.dma_start(out=out[n, :, h:h + nr, :],
                                          in_=ot[:])
                    else:
                        nc.scalar.activation(
                            out=ot[:], in_=ysrc, func=AF.Identity,
                            bias=bias_t[:, 0:1], scale=scale_t[:, 0:1])
                        nc.scalar.dma_start(out=out[n, :, h:h + nr, :],
                                            in_=ot[:])

    nc.compile()
    return nc


def kernel(x, weight, gamma, beta):
    x = np.asarray(x, dtype=np.float32)
    weight = np.asarray(weight, dtype=np.float32)
    gamma = np.asarray(gamma, dtype=np.float32)
    beta = np.asarray(beta, dtype=np.float32)

    if "nc" not in _CACHE:
        _CACHE["nc"] = _build()
    nc = _CACHE["nc"]

    # wt[ci, kh*3+kw, co] = sign(weight[co, ci, kh, kw]) as fp8 (+/-1 exact)
    import ml_dtypes
    wt = np.sign(
        np.ascontiguousarray(weight.transpose(1, 2, 3, 0)).reshape(C, 9, C)
    ).astype(ml_dtypes.float8_e4m3)
    gb = np.ascontiguousarray(np.stack([gamma, beta], axis=1))

    in_maps = []
    for i in range(N_CORES):
        in_maps.append({
            "x": np.ascontiguousarray(x[i * NIMG:(i + 1) * NIMG]),
            "wt": wt,
            "gb": gb,
        })

    res = bass_utils.run_bass_kernel_spmd(
        nc, in_maps, core_ids=list(range(N_CORES)), trace=TRACE)
    _CACHE["last_result"] = res

    out = np.empty((N_FULL, C, H, W), dtype=np.float32)
    for i in range(N_CORES):
        out[i * NIMG:(i + 1) * NIMG] = res.results[i]["out"]
    return out
